# revision 53
# baseline (speedup 1.0000x reference)
"""Trainium2 Bass kernel for an enhanced vector-quantizer (VQ codebook) module.

Contract: kernel(**inputs) takes the FULL inputs
    inputs     : [16, 4096, 256] float32
    emb_weight : [2048, 256]     float32
and returns the same tuple as the reference:
    (quantized_st [16,4096,256] f32, total_loss f32 scalar,
     perplexity f32 scalar, idx [16,4096] int32)

Strategy (8 NeuronCores, data-parallel over the flat token axis):
  - each core gets 8192 tokens; the [2048, 256] codebook is replicated.
  - nearest-code search via argmax of s = 2*x.e - |e|^2 computed on the PE
    with fp32r (11-bit mantissa) hi/lo split => 3 full-speed passes with
    fp64-level accuracy.
  - the -|e|^2 baseline is PRE-FILLED into each PSUM slot by the (idle)
    ScalarE: after a slot's first (seeded) accumulation group closes, its
    has_written bits stay set, so start=False matmuls accumulate onto the
    ACT-written baseline -- no per-tile e^2 matmuls on the PE at all.
  - argmax via DVE max + max_index on the SBUF copy (first-occurrence,
    matches argmin); the ACT PSUM->SBUF copy is the slot's only reader so
    Tile's bank-overlap tracker has nothing to serialize against.
  - quantized rows gathered from HBM with per-partition indirect DMA.
  - straight-through output st == q elementwise; mse finished on host.
  - per-core slice of the codebook self-similarity |sim| sum on the PE
    (fp32r) with the |.| row-sum on ScalarE (Abs + accumulate).
  - scalar losses / bincount / KL / perplexity finished on host from the
    exact per-core partials (the all-reduce step of the sharding hint).
"""

import sys

if "/opt/trn_rl_repo" not in sys.path:
    sys.path.insert(0, "/opt/trn_rl_repo")

import numpy as np

import concourse.bacc as bacc
import concourse.bass as bass
import concourse.mybir as mybir
from concourse.tile import TileContext
from concourse import bass_utils

F32 = mybir.dt.float32
F32R = mybir.dt.float32r
BF16 = mybir.dt.bfloat16
U32 = mybir.dt.uint32
I32 = mybir.dt.int32

N_CORES = 8
B, T_SEQ, D = 16, 4096, 256
K = 2048
N_TOKENS = B * T_SEQ          # 65536
T_CORE = N_TOKENS // N_CORES  # 8192 tokens per core
NT = T_CORE // 128            # 64 token tiles per core
KD = D // 128                 # 2 contraction chunks
NB = K // 512                 # 4 psum banks of codes

COMMITMENT_COST = 0.25
DIVERSITY_GAMMA = 0.1
EPS = 1e-8


def _round_f32r(a: np.ndarray) -> np.ndarray:
    """Round fp32 to the fp32r grid (11 mantissa bits, round-to-nearest)."""
    b = np.ascontiguousarray(a, dtype=np.float32).view(np.uint32)
    b2 = (b + np.uint32(1 << 11)) & np.uint32(0xFFFFF000)
    return b2.view(np.float32)


def _bf16_parts(a: np.ndarray, n: int) -> np.ndarray:
    """Decompose fp32 array into n bf16 summands (hi..lo)."""
    import ml_dtypes

    rem = a.astype(np.float32).copy()
    parts = []
    for _ in range(n):
        p = rem.astype(ml_dtypes.bfloat16)
        parts.append(p)
        rem = rem - p.astype(np.float32)
    return np.stack(parts, axis=0)


def _build_nc(nt: int = NT):
    t_core = nt * 128
    nc = bacc.Bacc("TRN2", target_bir_lowering=False, debug=False,
                   num_devices=N_CORES)

    # ---- DRAM I/O ----
    xh_T = nc.dram_tensor("xh_T", [nt, 128, KD, 128], F32R, kind="ExternalInput")
    xl_T = nc.dram_tensor("xl_T", [nt, 128, KD, 128], F32R, kind="ExternalInput")
    eh_T = nc.dram_tensor("eh_T", [128, KD, K], F32R, kind="ExternalInput")
    el_T = nc.dram_tensor("el_T", [128, KD, K], F32R, kind="ExternalInput")
    e2neg = nc.dram_tensor("e2neg", [3, K], BF16, kind="ExternalInput")
    nrm_T = nc.dram_tensor("nrm_T", [128, KD, K], F32R, kind="ExternalInput")
    nrmsl = nc.dram_tensor("nrmsl", [128, KD, 256], F32R, kind="ExternalInput")
    emb = nc.dram_tensor("emb", [K, D], F32, kind="ExternalInput")

    st_out = nc.dram_tensor("st_out", [t_core, D], F32, kind="ExternalOutput")
    idx_out = nc.dram_tensor("idx_out", [128, nt], I32, kind="ExternalOutput")
    simabs_out = nc.dram_tensor("simabs_out", [128, 2], F32, kind="ExternalOutput")

    with TileContext(nc) as tc:
        with tc.tile_pool(name="const", bufs=1) as cpool, \
             tc.tile_pool(name="xload", bufs=6) as xpool, \
             tc.tile_pool(name="work", bufs=3) as wpool, \
             tc.tile_pool(name="qwork", bufs=3) as qpool, \
             tc.tile_pool(name="psum", bufs=2, space="PSUM") as ppool:

            # resident constants
            eh_s = cpool.tile([128, KD, K], F32R)
            el_s = cpool.tile([128, KD, K], F32R)
            nrm_s = cpool.tile([128, KD, K], F32R)
            nrmsl_s = cpool.tile([128, KD, 256], F32R)
            e2_s = cpool.tile([3, K], BF16)
            ones_s = cpool.tile([3, 128], BF16)
            idxcols = cpool.tile([128, nt], U32)
            simabs_s = cpool.tile([128, 2], F32)
            e2rep = cpool.tile([128, K], F32)

            # operands needed by token tile 0 first; sim-phase operands last.
            # prefetch the first two token tiles ahead of the 4MB codebook
            # load so the PE can start as soon as bank 0 lands.
            nc.sync.dma_start(e2_s[:, :], e2neg.ap())
            # tile 0 first, then codebook bank 0, then deeper prefetch and
            # the remaining banks, so the first matmuls start ASAP.
            prefetch = []

            def _prefetch_tile(t):
                xh_t = xpool.tile([128, KD, 128], F32R, name=f"xh_p{t}", tag="xh_t")
                xl_t = xpool.tile([128, KD, 128], F32R, name=f"xl_p{t}", tag="xl_t")
                nc.sync.dma_start(xh_t[:, :, :], xh_T.ap()[t])
                nc.sync.dma_start(xl_t[:, :, :], xl_T.ap()[t])
                prefetch.append((xh_t, xl_t))

            def _load_bank(b_):
                cs = slice(b_ * 512, (b_ + 1) * 512)
                for k_ in range(KD):
                    nc.sync.dma_start(eh_s[:, k_, cs], eh_T.ap()[:, k_, cs])
                    nc.sync.dma_start(el_s[:, k_, cs], el_T.ap()[:, k_, cs])

            _prefetch_tile(0)
            _load_bank(0)
            if nt > 1:
                _prefetch_tile(1)
            _load_bank(1)
            if nt > 2:
                _prefetch_tile(2)
            for b_ in range(2, NB):
                _load_bank(b_)
            nc.vector.memset(ones_s[:, :], 1.0)

            # build the replicated -|e|^2 baseline once: ones.T @ e2neg.
            # Repeated twice with identical operands (idempotent -- the
            # last write wins and every write is the same): the extra rounds
            # run inside the codebook-DMA wait window and warm the PE HAM
            # clock gate so the first real distance matmuls run at 2.4GHz.
            pe2 = ppool.tile([128, K], F32, tag="pd", name="pe2")
            for rep in range(2):
                for b_ in range(NB):
                    cs = slice(b_ * 512, (b_ + 1) * 512)
                    nc.tensor.matmul(pe2[:, cs], ones_s[:, :], e2_s[:, cs],
                                     start=True, stop=True)
            nc.scalar.copy(e2rep[:, :], pe2[:, :])

            # ---- main distance/argmin/gather loop over token tiles ----
            for t in range(nt):
                ts = slice(t * 128, (t + 1) * 128)

                if t < len(prefetch):
                    xh_t, xl_t = prefetch[t]
                else:
                    xh_t = xpool.tile([128, KD, 128], F32R, tag="xh_t")
                    xl_t = xpool.tile([128, KD, 128], F32R, tag="xl_t")
                    nc.sync.dma_start(xh_t[:, :, :], xh_T.ap()[t])
                    nc.sync.dma_start(xl_t[:, :, :], xl_T.ap()[t])
                if t == 1:
                    # sim-phase operands, needed only at the very end
                    nc.sync.dma_start(nrm_s[:, :, :], nrm_T.ap())
                    nc.sync.dma_start(nrmsl_s[:, :, :], nrmsl.ap())

                pd = ppool.tile([128, K], F32, tag="pd")
                if t >= 2:
                    # this slot's has_written bits persist from its previous
                    # (closed) group: an ACT write sets the -|e|^2 baseline
                    # and the start=False matmuls accumulate onto it
                    nc.scalar.copy(pd[:, :], e2rep[:, :])
                sk = t >= 2
                for b_ in range(NB):
                    cs = slice(b_ * 512, (b_ + 1) * 512)
                    if t < 2:
                        # seed has_written bits + -|e|^2 via a real group
                        nc.tensor.matmul(pd[:, cs], ones_s[:, :], e2_s[:, cs],
                                         start=True, stop=False)
                    # identical stationary operands kept adjacent
                    nc.tensor.matmul(pd[:, cs], xh_t[:, 0, :], eh_s[:, 0, cs],
                                     start=False, stop=False, skip_group_check=sk)
                    nc.tensor.matmul(pd[:, cs], xh_t[:, 0, :], el_s[:, 0, cs],
                                     start=False, stop=False, skip_group_check=sk)
                    nc.tensor.matmul(pd[:, cs], xh_t[:, 1, :], eh_s[:, 1, cs],
                                     start=False, stop=False, skip_group_check=sk)
                    nc.tensor.matmul(pd[:, cs], xh_t[:, 1, :], el_s[:, 1, cs],
                                     start=False, stop=False, skip_group_check=sk)
                    nc.tensor.matmul(pd[:, cs], xl_t[:, 0, :], eh_s[:, 0, cs],
                                     start=False, stop=False, skip_group_check=sk)
                    nc.tensor.matmul(pd[:, cs], xl_t[:, 1, :], eh_s[:, 1, cs],
                                     start=False, stop=True, skip_group_check=sk)

                # distances to SBUF; the ACT copy is pd's ONLY reader so
                # the PSUM bank tracker has nothing to serialize against
                d_t = wpool.tile([128, K], F32)
                nc.scalar.copy(d_t[:, :], pd[:, :])
                mx8_t = wpool.tile([128, 8], F32)
                nc.vector.max(mx8_t[:, :], d_t[:, :])
                mi = wpool.tile([128, 8], U32)
                nc.vector.max_index(mi[:, :], mx8_t[:, :], d_t[:, :])
                nc.vector.tensor_copy(idxcols[:, t:t + 1], mi[:, 0:1])

                # gather the selected code rows
                q_t = qpool.tile([128, D], F32)
                nc.gpsimd.indirect_dma_start(
                    out=q_t[:, :], out_offset=None,
                    in_=emb.ap(),
                    in_offset=bass.IndirectOffsetOnAxis(ap=mi[:, 0:1], axis=0),
                )

                # straight-through output st = x + (q - x) == q elementwise;
                # the mse partial sums are finished on the host from st_out
                nc.sync.dma_start(st_out.ap()[ts, :], q_t[:, :])

            # ---- codebook self-similarity slice: |normed @ normed.T| ----
            # this core's 256 rows (host passes the slice) x all 2048 codes;
            # runs at the end so the PE tail overlaps the last argmin work.
            # The |.| row-sum runs on the (idle) ScalarE via Abs + accum.
            simtrash = cpool.tile([128, K], F32)
            for m in range(2):
                psim = ppool.tile([128, K], F32, tag="pd", name=f"psim{m}")
                for b_ in range(NB):
                    cs = slice(b_ * 512, (b_ + 1) * 512)
                    nc.tensor.matmul(psim[:, cs], nrmsl_s[:, 0, m * 128:(m + 1) * 128],
                                     nrm_s[:, 0, cs], start=True, stop=False)
                    nc.tensor.matmul(psim[:, cs], nrmsl_s[:, 1, m * 128:(m + 1) * 128],
                                     nrm_s[:, 1, cs], start=False, stop=True)
                nc.scalar.activation(simtrash[:, :], psim[:, :],
                                     mybir.ActivationFunctionType.Abs,
                                     accum_out=simabs_s[:, m:m + 1])
            nc.sync.dma_start(simabs_out.ap(), simabs_s[:, :])

            # ---- tail: write idx ----
            nc.sync.dma_start(idx_out.ap(), idxcols[:, :].bitcast(I32))

    nc.compile()
    return nc


_NC_CACHE = {}


def _get_nc(nt: int = NT):
    if nt not in _NC_CACHE:
        _NC_CACHE[nt] = _build_nc(nt)
    return _NC_CACHE[nt]


def _prep_in_maps(inputs: np.ndarray, emb_weight: np.ndarray):
    x = np.ascontiguousarray(inputs, dtype=np.float32).reshape(N_TOKENS, D)
    e = np.ascontiguousarray(emb_weight, dtype=np.float32)

    # codebook-side operands (shared by all cores)
    e2 = (e.astype(np.float64) ** 2).sum(axis=1)
    e2neg = _bf16_parts(-e2.astype(np.float32), 3)          # [3, K]
    eT2 = np.ascontiguousarray((2.0 * e).T)                 # [D, K]
    eh = _round_f32r(eT2)
    el = _round_f32r(eT2 - eh)
    eh = eh.reshape(KD, 128, K).transpose(1, 0, 2)          # [128, KD, K]
    el = el.reshape(KD, 128, K).transpose(1, 0, 2)

    row_norm = np.sqrt((e.astype(np.float32) ** 2).sum(axis=1))
    normed = e / np.maximum(row_norm, 1e-12)[:, None]
    nT = _round_f32r(np.ascontiguousarray(normed.T))        # [D, K]
    nrm = nT.reshape(KD, 128, K).transpose(1, 0, 2)         # [128, KD, K]

    eh = np.ascontiguousarray(eh)
    el = np.ascontiguousarray(el)
    nrm = np.ascontiguousarray(nrm)

    in_maps = []
    nt = T_CORE // 128
    for c in range(N_CORES):
        xs = x[c * T_CORE:(c + 1) * T_CORE]                 # [T_CORE, D]
        xT = np.ascontiguousarray(xs.T)                     # [D, T_CORE]
        xh = _round_f32r(xT)
        xl = _round_f32r(xT - xh)
        # [D, T] -> [nt, 128(p), KD, 128(m)]: element (a*128+p, t*128+m)
        xh = np.ascontiguousarray(
            xh.reshape(KD, 128, nt, 128).transpose(2, 1, 0, 3))
        xl = np.ascontiguousarray(
            xl.reshape(KD, 128, nt, 128).transpose(2, 1, 0, 3))
        nsl = np.ascontiguousarray(nT[:, c * 256:(c + 1) * 256]
                                   .reshape(KD, 128, 256).transpose(1, 0, 2))
        in_maps.append({
            "xh_T": xh, "xl_T": xl,
            "eh_T": eh, "el_T": el, "e2neg": np.ascontiguousarray(e2neg),
            "nrm_T": nrm, "nrmsl": nsl, "emb": e,
        })
    return in_maps, row_norm


def kernel(inputs: np.ndarray, emb_weight: np.ndarray):
    inputs = np.asarray(inputs)
    emb_weight = np.asarray(emb_weight)
    nc = _get_nc()
    in_maps, row_norm = _prep_in_maps(inputs, emb_weight)
    res = bass_utils.run_bass_kernel_spmd(nc, in_maps,
                                          core_ids=list(range(N_CORES)))

    sts, idxs = [], []
    simabs_total = 0.0
    for om in res.results:
        sts.append(om["st_out"])
        # idx_out is [128, nt] partition-major: token t*128+p at [p, t]
        idxs.append(om["idx_out"].T.reshape(-1))
        simabs_total += float(om["simabs_out"].astype(np.float64).sum())

    quantized_st = np.concatenate(sts, axis=0).reshape(B, T_SEQ, D)
    idx = np.concatenate(idxs, axis=0).astype(np.int32)

    # sse = sum((q - x)^2) finished on host from the gathered output
    qf = quantized_st.reshape(-1, D)
    xf = np.asarray(inputs, dtype=np.float32).reshape(-1, D)
    df = qf.astype(np.float64) - xf.astype(np.float64)
    sse_total = float((df * df).sum())

    # ---- host-side finishing (all-reduce of scalars + bincount) ----
    counts = np.bincount(idx, minlength=K).astype(np.float64)
    total = counts.sum()

    mse = sse_total / (N_TOKENS * D)
    vq_loss = mse + COMMITMENT_COST * mse

    p = counts / total
    t_u = 1.0 / K
    kl = float(np.sum((p + EPS) * np.log((p + EPS) / (t_u + EPS))))
    kl = min(kl, 100.0)

    l2_reg = min(float(row_norm.astype(np.float64).mean()), 10.0)
    orth_reg = min((simabs_total - K) / (K * K), 10.0)
    reg_loss = l2_reg + orth_reg

    total_loss = np.float32(min(vq_loss + DIVERSITY_GAMMA * kl
                                + 0.01 * reg_loss, 100.0))

    nz = p > 0
    perplexity = np.float32(np.exp(-np.sum(p[nz] * np.log(p[nz]))))

    return (quantized_st, total_loss, perplexity,
            idx.reshape(B, T_SEQ).astype(np.int32))
